# revision 10
# baseline (speedup 1.0000x reference)
"""Trainium2 Bass kernel for ExtGNNLayer message passing.

kernel(**inputs) -> (ent_new, rel_new), matching the reference:
    comp_h = concat([rel_emb[etypes], ent_emb[src]])
    msg    = where(etypes < NUM_REL, comp_h @ W_O.T + b_O, comp_h @ W_I.T + b_I)
    h_agg  = segment_mean(msg, dst)
    ent_new = ent_emb @ W_S.T + b_S + h_agg
    rel_new = rel_emb @ W_R.T + b_R

Distribution: edges sharded by destination-node ownership across 8 cores
(6250 nodes/core) -> fully independent cores, no collectives.

Per-core device algorithm (linear algebra reassociated so the per-edge
weight matmuls become per-window):
  - rel_part[t] = rel_emb[t] @ W1_sel(t).T + b_sel(t) precomputed host-side
    (weight folding; W1_sel = first 128 cols of W_O/W_I).
  - Edge rows of ent_emb and rel_part are fetched with batched indirect
    DMA gathers: one SWDGE call per superblock, idx[p, j] -> row at
    dest[p, j*128:(j+1)*128].
  - For each 128-dst-node window, per 128-edge tile (edge e = partition):
      M2[e, n + 128*is_inv_e] = recip_deg[dst_e] * (dst_off_e == n)  (1 DVE op)
      psumA  += entg_tile.T(k=e) @ M2     -> A_O | A_I  [c, 256]
      psumRel+= relg_tile.T(k=e) @ M2     -> R_O | R_I  [d, 256]
  - Window finish:
      psumRel[:, :128] += W_O2 @ A_O + W_I2 @ A_I      (2 matmuls)
      out[d, n] = psumRel_O + psumRel_I + b_S[d] + (W_S @ entT_win)[d, n]
  - Outputs are written transposed [d, n]; the host transposes back.
"""

import dataclasses
import math

import numpy as np

import concourse.bacc as bacc
import concourse.bass as bass
import concourse.mybir as mybir
import concourse.tile as tile
from concourse.bass_utils import run_bass_kernel_spmd

P = 128
F32 = mybir.dt.float32
I32 = mybir.dt.int32
ADD = mybir.AluOpType.add
MULT = mybir.AluOpType.mult
ISEQ = mybir.AluOpType.is_equal


@dataclasses.dataclass
class Cfg:
    n_nodes: int = 50000
    n_rel: int = 200            # etypes < n_rel -> W_O path, else W_I
    dim: int = 128
    n_cores: int = 8
    sb_w: int = 2               # windows per superblock (per gather call)

    @property
    def n_rel2(self):
        return 2 * self.n_rel

    @property
    def npc(self):
        return self.n_nodes // self.n_cores

    @property
    def n_win(self):
        return math.ceil(self.npc / P)


@dataclasses.dataclass
class Plan:
    cfg: Cfg
    t_w: np.ndarray           # [n_win] common (max-over-core) tile counts
    win_t0: dict              # w -> first global tile index
    n_tiles: int


def _make_plan(counts, cfg: Cfg) -> Plan:
    """counts: [n_cores, n_win] edge counts."""
    t_w = np.maximum(np.ceil(counts.max(axis=0) / P).astype(np.int64), 0)
    win_t0 = {}
    col = 0
    for w in range(cfg.n_win):
        win_t0[w] = col
        col += int(t_w[w])
    return Plan(cfg=cfg, t_w=t_w, win_t0=win_t0, n_tiles=col)


def _pack_core(plan: Plan, cfg: Cfg, core: int, src, dst, etypes, recip_deg,
               edge_order, block_bounds):
    """Build this core's device arrays ([128, NT] layouts; edge (tile t,
    partition p) at column t, row p)."""
    NT = plan.n_tiles
    ent_idx = np.zeros(NT * P, dtype=np.int32)
    rel_idx = np.zeros(NT * P, dtype=np.int32)
    dstmod = np.full(NT * P, -1.0, dtype=np.float32)
    rd = np.zeros(NT * P, dtype=np.float32)

    W = cfg.n_win
    for w in range(W):
        nt = int(plan.t_w[w])
        if nt == 0:
            continue
        b = core * W + w
        e0, e1 = block_bounds[b], block_bounds[b + 1]
        eids = edge_order[e0:e1]
        cnt = len(eids)
        assert cnt <= nt * P
        s = plan.win_t0[w] * P
        ent_idx[s:s + cnt] = src[eids]
        rel_idx[s:s + cnt] = etypes[eids]
        n_off = (dst[eids] - core * cfg.npc - w * P).astype(np.float32)
        is_inv = (etypes[eids] >= cfg.n_rel).astype(np.float32)
        dstmod[s:s + cnt] = n_off + P * is_inv
        rd[s:s + cnt] = recip_deg[dst[eids]]

    return dict(
        ent_idx=ent_idx.reshape(NT, P).T.copy(),
        rel_idx=rel_idx.reshape(NT, P).T.copy(),
        dstmod=dstmod.reshape(NT, P).T.copy(),
        rd=rd.reshape(NT, P).T.copy(),
    )


def _host_prep(inputs, cfg: Cfg):
    src = np.ascontiguousarray(np.asarray(inputs["src"]).astype(np.int64))
    dst = np.ascontiguousarray(np.asarray(inputs["dst"]).astype(np.int64))
    etypes = np.ascontiguousarray(np.asarray(inputs["etypes"]).astype(np.int64))
    ent_emb = np.asarray(inputs["ent_emb"], dtype=np.float32)
    rel_emb = np.asarray(inputs["rel_emb"], dtype=np.float32)
    W_O_w = np.asarray(inputs["W_O_w"], dtype=np.float32)
    W_O_b = np.asarray(inputs["W_O_b"], dtype=np.float32)
    W_I_w = np.asarray(inputs["W_I_w"], dtype=np.float32)
    W_I_b = np.asarray(inputs["W_I_b"], dtype=np.float32)
    W_S_w = np.asarray(inputs["W_S_w"], dtype=np.float32)
    W_S_b = np.asarray(inputs["W_S_b"], dtype=np.float32)
    W_R_w = np.asarray(inputs["W_R_w"], dtype=np.float32)
    W_R_b = np.asarray(inputs["W_R_b"], dtype=np.float32)

    D = cfg.dim
    deg = np.bincount(dst, minlength=cfg.n_nodes).astype(np.float32)
    recip_deg = (1.0 / np.maximum(deg, 1.0)).astype(np.float32)

    # rel_part[t] = rel_emb[t] @ W1_sel.T + b_sel  (weight folding)
    rel_part_o = rel_emb @ W_O_w[:, :D].T + W_O_b
    rel_part_i = rel_emb @ W_I_w[:, :D].T + W_I_b
    is_inv_t = (np.arange(cfg.n_rel2) >= cfg.n_rel)[:, None]
    rel_part = np.where(is_inv_t, rel_part_i, rel_part_o).astype(np.float32)

    # edge blocks sorted by (core, window), then by src inside each block
    # (ascending gather addresses -> better HBM locality)
    core = dst // cfg.npc
    w = (dst - core * cfg.npc) // P
    key = core * cfg.n_win + w
    edge_order = np.lexsort((src, key))
    counts_flat = np.bincount(key, minlength=cfg.n_cores * cfg.n_win)
    block_bounds = np.concatenate([[0], np.cumsum(counts_flat)])
    counts = counts_flat.reshape(cfg.n_cores, cfg.n_win)

    plan = _make_plan(counts, cfg)

    WN = cfg.n_win * P
    common = dict(
        ent_emb=ent_emb,
        rel_part=rel_part,
        iota=np.broadcast_to(np.arange(256, dtype=np.float32),
                             (P, 256)).copy(),
        W_O2T=W_O_w[:, D:].T.copy(),
        W_I2T=W_I_w[:, D:].T.copy(),
        W_ST=W_S_w.T.copy(),
        b_col=W_S_b[:, None].copy(),
        rel_embT=np.zeros((P, cfg.n_rel2), dtype=np.float32),
        W_RT=W_R_w.T.copy(),
        W_R_bcol=W_R_b[:, None].copy(),
    )
    common["rel_embT"][:rel_emb.shape[1], :] = rel_emb.T

    in_maps = []
    for c in range(cfg.n_cores):
        pc = _pack_core(plan, cfg, c, src, dst, etypes, recip_deg,
                        edge_order, block_bounds)
        entT = np.zeros((P, WN), dtype=np.float32)
        sl = ent_emb[c * cfg.npc:(c + 1) * cfg.npc]
        entT[:, :sl.shape[0]] = sl.T
        m = dict(common)
        m.update(pc)
        m["entT"] = entT
        in_maps.append(m)
    return plan, in_maps


def _build_kernel(plan: Plan, repeat: int = 1, num_devices: int | None = None):
    cfg = plan.cfg
    D = cfg.dim
    NT = plan.n_tiles
    WN = cfg.n_win * P

    nc = bacc.Bacc("TRN2", target_bir_lowering=False, debug=False,
                   num_devices=num_devices or cfg.n_cores)

    dr = {}
    def din(name, shape, dt=F32):
        dr[name] = nc.dram_tensor(name, shape, dt, kind="ExternalInput").ap()

    din("ent_emb", [cfg.n_nodes, D])
    din("rel_part", [cfg.n_rel2, D])
    din("iota", [P, 256])
    din("W_O2T", [D, D])
    din("W_I2T", [D, D])
    din("W_ST", [D, D])
    din("b_col", [D, 1])
    din("rel_embT", [P, cfg.n_rel2])
    din("W_RT", [D, D])
    din("W_R_bcol", [D, 1])
    din("entT", [P, WN])
    din("ent_idx", [P, NT], I32)
    din("rel_idx", [P, NT], I32)
    din("dstmod", [P, NT])
    din("rd", [P, NT])

    ent_newT = nc.dram_tensor("ent_newT", [P, WN], F32,
                              kind="ExternalOutput").ap()
    rel_newT = nc.dram_tensor("rel_newT", [P, cfg.n_rel2], F32,
                              kind="ExternalOutput").ap()

    with tile.TileContext(nc) as tc:
        with (
            tc.tile_pool(name="const", bufs=1) as cpool,
            tc.tile_pool(name="entg", bufs=8) as entg_pool,
            tc.tile_pool(name="relg", bufs=8) as relg_pool,
            tc.tile_pool(name="m2", bufs=8) as m2_pool,
            tc.tile_pool(name="fin", bufs=3) as fin_pool,
            tc.tile_pool(name="psA", bufs=2, space="PSUM") as psA_pool,
            tc.tile_pool(name="psR", bufs=2, space="PSUM") as psR_pool,
            tc.tile_pool(name="psF", bufs=2, space="PSUM") as psF_pool,
        ):
            def load_const(name, shape, dt=F32):
                t = cpool.tile(shape, dt, tag=name)
                nc.sync.dma_start(t[:], dr[name][:])
                return t

            iota = load_const("iota", [P, 256])
            W_O2T = load_const("W_O2T", [D, D])
            W_I2T = load_const("W_I2T", [D, D])
            W_ST = load_const("W_ST", [D, D])
            b_col = load_const("b_col", [D, 1])
            rel_embT = load_const("rel_embT", [P, cfg.n_rel2])
            W_RT = load_const("W_RT", [D, D])
            W_R_bcol = load_const("W_R_bcol", [D, 1])
            entT = load_const("entT", [P, WN])
            ent_idx = load_const("ent_idx", [P, NT], I32)
            rel_idx = load_const("rel_idx", [P, NT], I32)
            dstmod = load_const("dstmod", [P, NT])
            rd = load_const("rd", [P, NT])

            # ---- rel_new = (W_R @ rel_embT + b) transposed ----
            ps_rn = psF_pool.tile([P, 512], F32, tag="psrn")
            n_rt = math.ceil(cfg.n_rel2 / P)
            for j in range(n_rt):
                c0 = j * P
                c1 = min(c0 + P, cfg.n_rel2)
                nc.tensor.matmul(ps_rn[:, c0:c1], lhsT=W_RT[:],
                                 rhs=rel_embT[:, c0:c1], start=True, stop=True)
            sb_rn = fin_pool.tile([P, cfg.n_rel2], F32, tag="sbrn")
            nc.vector.tensor_scalar(sb_rn[:], ps_rn[:, :cfg.n_rel2],
                                    W_R_bcol[:], None, ADD)
            nc.sync.dma_start(rel_newT[:], sb_rn[:])

            # ---- main loop over windows ----
            for w in [w for _ in range(repeat) for w in range(cfg.n_win)]:
                    nt_w = int(plan.t_w[w])
                    psF = psF_pool.tile([P, P], F32, tag="psF")
                    nc.tensor.matmul(psF[:], lhsT=W_ST[:],
                                     rhs=entT[:, w * P:(w + 1) * P],
                                     start=True, stop=True)
                    if nt_w:
                        psA = psA_pool.tile([P, 256], F32, tag="psA")
                        psR = psR_pool.tile([P, 256], F32, tag="psR")
                        for i in range(nt_w):
                            col = plan.win_t0[w] + i
                            eg = entg_pool.tile([P, D], F32, tag="entg")
                            nc.gpsimd.indirect_dma_start(
                                out=eg[:], out_offset=None,
                                in_=dr["ent_emb"][:],
                                in_offset=bass.IndirectOffsetOnAxis(
                                    ap=ent_idx[:, col:col + 1], axis=0))
                            rg = relg_pool.tile([P, D], F32, tag="relg")
                            nc.gpsimd.indirect_dma_start(
                                out=rg[:], out_offset=None,
                                in_=dr["rel_part"][:],
                                in_offset=bass.IndirectOffsetOnAxis(
                                    ap=rel_idx[:, col:col + 1], axis=0))
                            m2 = m2_pool.tile([P, 256], F32, tag="m2")
                            nc.vector.scalar_tensor_tensor(
                                out=m2[:],
                                in0=iota[:],
                                scalar=dstmod[:, col:col + 1],
                                in1=rd[:, col:col + 1].to_broadcast([P, 256]),
                                op0=ISEQ, op1=MULT)
                            first = (i == 0)
                            last = (i == nt_w - 1)
                            nc.tensor.matmul(psA[:], lhsT=eg[:],
                                             rhs=m2[:], start=first, stop=last)
                            nc.tensor.matmul(psR[:], lhsT=rg[:],
                                             rhs=m2[:], start=first, stop=False)
                        sbA = fin_pool.tile([P, 256], F32, tag="sbA")
                        nc.vector.tensor_copy(sbA[:], psA[:])
                        nc.tensor.matmul(psR[:, :P], lhsT=W_O2T[:],
                                         rhs=sbA[:, :P], start=False,
                                         stop=False)
                        nc.tensor.matmul(psR[:, :P], lhsT=W_I2T[:],
                                         rhs=sbA[:, P:], start=False, stop=True)
                        tmpI = fin_pool.tile([P, P], F32, tag="tmpI")
                        nc.vector.tensor_copy(tmpI[:], psR[:, P:])
                        s1 = fin_pool.tile([P, P], F32, tag="s1")
                        nc.vector.tensor_tensor(s1[:], psR[:, :P], tmpI[:], ADD)
                        outw = fin_pool.tile([P, P], F32, tag="outw")
                        nc.vector.scalar_tensor_tensor(
                            out=outw[:], in0=s1[:], scalar=b_col[:],
                            in1=psF[:], op0=ADD, op1=ADD)
                    else:
                        outw = fin_pool.tile([P, P], F32, tag="outw")
                        nc.vector.tensor_scalar(outw[:], psF[:], b_col[:],
                                                None, ADD)
                    nc.sync.dma_start(ent_newT[:, w * P:(w + 1) * P], outw[:])

    nc.compile()
    return nc


def _run(inputs, cfg: Cfg, trace=False):
    plan, in_maps = _host_prep(inputs, cfg)
    nc = _build_kernel(plan)
    res = run_bass_kernel_spmd(nc, in_maps, core_ids=list(range(cfg.n_cores)),
                               trace=trace)
    ent_rows = []
    for c in range(cfg.n_cores):
        ent_rows.append(res.results[c]["ent_newT"][:, :cfg.npc].T)
    ent_new = np.ascontiguousarray(np.concatenate(ent_rows, axis=0))
    rel_new = np.ascontiguousarray(
        res.results[0]["rel_newT"][:, :cfg.n_rel2].T)
    return (ent_new, rel_new), res


def kernel(**inputs):
    cfg = Cfg()
    (ent_new, rel_new), _ = _run(inputs, cfg)
    return ent_new, rel_new


# revision 20
# speedup vs baseline: 2.1270x; 2.1270x over previous
"""Trainium2 Bass kernel for ExtGNNLayer message passing.

kernel(**inputs) -> (ent_new, rel_new), matching the reference:
    comp_h = concat([rel_emb[etypes], ent_emb[src]])
    msg    = where(etypes < NUM_REL, comp_h @ W_O.T + b_O, comp_h @ W_I.T + b_I)
    h_agg  = segment_mean(msg, dst)
    ent_new = ent_emb @ W_S.T + b_S + h_agg
    rel_new = rel_emb @ W_R.T + b_R

Distribution: edges sharded by destination-node ownership across 8 cores
(6250 nodes/core) -> fully independent cores, no collectives.

Per-core device algorithm (linear algebra reassociated so the per-edge
weight matmuls become per-window):
  - rel_part[t] = rel_emb[t] @ W1_sel(t).T + b_sel(t) precomputed host-side
    (weight folding; W1_sel = first 128 cols of W_O/W_I).
  - Edge rows of ent_emb and rel_part are fetched with batched indirect
    DMA gathers: one SWDGE call per superblock, idx[p, j] -> row at
    dest[p, j*128:(j+1)*128].
  - For each 128-dst-node window, per 128-edge tile (edge e = partition):
      M2[e, n + 128*is_inv_e] = recip_deg[dst_e] * (dst_off_e == n)  (1 DVE op)
      psumA  += entg_tile.T(k=e) @ M2     -> A_O | A_I  [c, 256]
      psumRel+= relg_tile.T(k=e) @ M2     -> R_O | R_I  [d, 256]
  - Window finish:
      psumRel[:, :128] += W_O2 @ A_O + W_I2 @ A_I      (2 matmuls)
      out[d, n] = psumRel_O + psumRel_I + b_S[d] + (W_S @ entT_win)[d, n]
  - Outputs are written transposed [d, n]; the host transposes back.
"""

import dataclasses
import math

import numpy as np

import concourse.bacc as bacc
import concourse.bass as bass
import concourse.mybir as mybir
import concourse.tile as tile
from concourse.bass_utils import run_bass_kernel_spmd

P = 128
F32 = mybir.dt.float32
I32 = mybir.dt.int32
ADD = mybir.AluOpType.add
MULT = mybir.AluOpType.mult
ISEQ = mybir.AluOpType.is_equal


@dataclasses.dataclass
class Cfg:
    n_nodes: int = 50000
    n_rel: int = 200            # etypes < n_rel -> W_O path, else W_I
    dim: int = 128
    n_cores: int = 8
    sb_w: int = 2               # windows per superblock (per gather call)

    @property
    def n_rel2(self):
        return 2 * self.n_rel

    @property
    def npc(self):
        return self.n_nodes // self.n_cores

    @property
    def n_win(self):
        return math.ceil(self.npc / P)


@dataclasses.dataclass
class Plan:
    cfg: Cfg
    t_w: np.ndarray           # [n_win] common (max-over-core) tile counts
    win_t0: dict              # w -> first global tile index
    n_tiles: int


def _make_plan(counts, cfg: Cfg) -> Plan:
    """counts: [n_cores, n_win] edge counts."""
    t_w = np.maximum(np.ceil(counts.max(axis=0) / P).astype(np.int64), 0)
    win_t0 = {}
    col = 0
    for w in range(cfg.n_win):
        win_t0[w] = col
        col += int(t_w[w])
    return Plan(cfg=cfg, t_w=t_w, win_t0=win_t0, n_tiles=col)


def _pack_core(plan: Plan, cfg: Cfg, core: int, src, dst, etypes, recip_deg,
               edge_order, block_bounds):
    """Build this core's device arrays ([128, NT] layouts; edge (tile t,
    partition p) at column t, row p)."""
    NT = plan.n_tiles
    ent_idx = np.zeros(NT * P, dtype=np.int32)
    dstmod = np.full(NT * P, -1.0, dtype=np.float32)
    rd = np.zeros(NT * P, dtype=np.float32)

    W = cfg.n_win
    WN = W * P
    TB = 4 * P  # padded relation-type dimension (4 blocks of 128)
    cw_flat = np.zeros(WN * TB, dtype=np.float32)
    for w in range(W):
        nt = int(plan.t_w[w])
        b = core * W + w
        e0, e1 = block_bounds[b], block_bounds[b + 1]
        eids = edge_order[e0:e1]
        cnt = len(eids)
        if cnt == 0:
            continue
        assert nt * P >= cnt
        s = plan.win_t0[w] * P
        ent_idx[s:s + cnt] = src[eids]
        n_off = (dst[eids] - core * cfg.npc - w * P).astype(np.float32)
        is_inv = (etypes[eids] >= cfg.n_rel).astype(np.float32)
        dstmod[s:s + cnt] = n_off + P * is_inv
        rdv = recip_deg[dst[eids]]
        rd[s:s + cnt] = rdv
        # scaled (node, type) counts for this window
        bins = (dst[eids] - core * cfg.npc).astype(np.int64) * TB + etypes[eids]
        np.add.at(cw_flat, bins, rdv)

    # C[w, p=t_local, blk*128+n] = sum_{edges->(w,n)} rd * 1[etype=blk*128+p]
    C = cw_flat.reshape(W, P, 4, P)          # [w, n, blk, t_local]
    C = C.transpose(0, 3, 2, 1).copy()       # [w, t_local, blk, n]

    return dict(
        ent_idx=ent_idx.reshape(NT, P).T.copy(),
        dstmod=dstmod.reshape(NT, P).T.copy(),
        rd=rd.reshape(NT, P).T.copy(),
        Cw=C.reshape(W, P, 4 * P),
    )


def _host_prep(inputs, cfg: Cfg):
    src = np.ascontiguousarray(np.asarray(inputs["src"]).astype(np.int64))
    dst = np.ascontiguousarray(np.asarray(inputs["dst"]).astype(np.int64))
    etypes = np.ascontiguousarray(np.asarray(inputs["etypes"]).astype(np.int64))
    ent_emb = np.asarray(inputs["ent_emb"], dtype=np.float32)
    rel_emb = np.asarray(inputs["rel_emb"], dtype=np.float32)
    W_O_w = np.asarray(inputs["W_O_w"], dtype=np.float32)
    W_O_b = np.asarray(inputs["W_O_b"], dtype=np.float32)
    W_I_w = np.asarray(inputs["W_I_w"], dtype=np.float32)
    W_I_b = np.asarray(inputs["W_I_b"], dtype=np.float32)
    W_S_w = np.asarray(inputs["W_S_w"], dtype=np.float32)
    W_S_b = np.asarray(inputs["W_S_b"], dtype=np.float32)
    W_R_w = np.asarray(inputs["W_R_w"], dtype=np.float32)
    W_R_b = np.asarray(inputs["W_R_b"], dtype=np.float32)

    D = cfg.dim
    deg = np.bincount(dst, minlength=cfg.n_nodes).astype(np.float32)
    recip_deg = (1.0 / np.maximum(deg, 1.0)).astype(np.float32)

    # rel_part[t] = rel_emb[t] @ W1_sel.T + b_sel  (weight folding)
    rel_part_o = rel_emb @ W_O_w[:, :D].T + W_O_b
    rel_part_i = rel_emb @ W_I_w[:, :D].T + W_I_b
    is_inv_t = (np.arange(cfg.n_rel2) >= cfg.n_rel)[:, None]
    rel_part = np.where(is_inv_t, rel_part_i, rel_part_o).astype(np.float32)

    # edge blocks sorted by (core, window), then by src inside each block
    # (ascending gather addresses -> better HBM locality)
    core = dst // cfg.npc
    w = (dst - core * cfg.npc) // P
    key = core * cfg.n_win + w
    edge_order = np.lexsort((src, key))
    counts_flat = np.bincount(key, minlength=cfg.n_cores * cfg.n_win)
    block_bounds = np.concatenate([[0], np.cumsum(counts_flat)])
    counts = counts_flat.reshape(cfg.n_cores, cfg.n_win)

    plan = _make_plan(counts, cfg)

    # rel_partP[p, blk*128+d] = rel_part[blk*128+p, d], zero-padded to 512 rows
    rel_partP = np.zeros((P, 4 * P), dtype=np.float32)
    for blk in range(4):
        rows = rel_part[blk * P:(blk + 1) * P]
        rel_partP[:rows.shape[0], blk * P:blk * P + P] = rows

    WN = cfg.n_win * P
    common = dict(
        ent_emb=ent_emb,
        rel_partP=rel_partP,
        iota=np.broadcast_to(np.arange(256, dtype=np.float32),
                             (P, 256)).copy(),
        W_O2T=W_O_w[:, D:].T.copy(),
        W_I2T=W_I_w[:, D:].T.copy(),
        W_ST=W_S_w.T.copy(),
        b_col=W_S_b[:, None].copy(),
        rel_embT=np.zeros((P, cfg.n_rel2), dtype=np.float32),
        W_RT=W_R_w.T.copy(),
        W_R_bcol=W_R_b[:, None].copy(),
    )
    common["rel_embT"][:rel_emb.shape[1], :] = rel_emb.T

    in_maps = []
    for c in range(cfg.n_cores):
        pc = _pack_core(plan, cfg, c, src, dst, etypes, recip_deg,
                        edge_order, block_bounds)
        entT = np.zeros((P, WN), dtype=np.float32)
        sl = ent_emb[c * cfg.npc:(c + 1) * cfg.npc]
        entT[:, :sl.shape[0]] = sl.T
        m = dict(common)
        m.update(pc)
        m["entT"] = entT
        in_maps.append(m)
    return plan, in_maps


def _build_kernel(plan: Plan, repeat: int = 1, num_devices: int | None = None):
    cfg = plan.cfg
    D = cfg.dim
    NT = plan.n_tiles
    WN = cfg.n_win * P

    nc = bacc.Bacc("TRN2", target_bir_lowering=False, debug=False,
                   num_devices=num_devices or cfg.n_cores,
                   dynamic_dma_scratch_size=65536)

    dr = {}
    def din(name, shape, dt=F32):
        dr[name] = nc.dram_tensor(name, shape, dt, kind="ExternalInput").ap()

    din("ent_emb", [cfg.n_nodes, D])
    din("rel_partP", [P, 4 * P])
    din("Cw", [cfg.n_win, P, 4 * P])
    din("iota", [P, 256])
    din("W_O2T", [D, D])
    din("W_I2T", [D, D])
    din("W_ST", [D, D])
    din("b_col", [D, 1])
    din("rel_embT", [P, cfg.n_rel2])
    din("W_RT", [D, D])
    din("W_R_bcol", [D, 1])
    din("entT", [P, WN])
    din("ent_idx", [P, NT], I32)
    din("dstmod", [P, NT])
    din("rd", [P, NT])

    ent_newT = nc.dram_tensor("ent_newT", [P, WN], F32,
                              kind="ExternalOutput").ap()
    rel_newT = nc.dram_tensor("rel_newT", [P, cfg.n_rel2], F32,
                              kind="ExternalOutput").ap()

    with tile.TileContext(nc) as tc:
        with (
            tc.tile_pool(name="const", bufs=1) as cpool,
            tc.tile_pool(name="entg", bufs=16) as entg_pool,
            tc.tile_pool(name="m2", bufs=12) as m2_pool,
            tc.tile_pool(name="cw", bufs=3) as cw_pool,
            tc.tile_pool(name="fin", bufs=3) as fin_pool,
            tc.tile_pool(name="psA", bufs=3, space="PSUM") as psA_pool,
            tc.tile_pool(name="psR", bufs=3, space="PSUM") as psR_pool,
            tc.tile_pool(name="psrn", bufs=1, space="PSUM") as psrn_pool,
        ):
            def load_const(name, shape, dt=F32):
                t = cpool.tile(shape, dt, tag=name)
                nc.sync.dma_start(t[:], dr[name][:])
                return t

            iota = load_const("iota", [P, 256])
            W_O2T = load_const("W_O2T", [D, D])
            W_I2T = load_const("W_I2T", [D, D])
            W_ST = load_const("W_ST", [D, D])
            b_col = load_const("b_col", [D, 1])
            rel_embT = load_const("rel_embT", [P, cfg.n_rel2])
            W_RT = load_const("W_RT", [D, D])
            W_R_bcol = load_const("W_R_bcol", [D, 1])
            rel_partP = load_const("rel_partP", [P, 4 * P])
            entT = load_const("entT", [P, WN])
            ent_idx = load_const("ent_idx", [P, NT], I32)
            dstmod = load_const("dstmod", [P, NT])
            rd = load_const("rd", [P, NT])

            # ---- rel_new = (W_R @ rel_embT + b) transposed ----
            ps_rn = psrn_pool.tile([P, 512], F32, tag="psrn")
            n_rt = math.ceil(cfg.n_rel2 / P)
            for j in range(n_rt):
                c0 = j * P
                c1 = min(c0 + P, cfg.n_rel2)
                nc.tensor.matmul(ps_rn[:, c0:c1], lhsT=W_RT[:],
                                 rhs=rel_embT[:, c0:c1], start=True, stop=True)
            sb_rn = fin_pool.tile([P, cfg.n_rel2], F32, tag="sbrn")
            nc.vector.tensor_scalar(sb_rn[:], ps_rn[:, :cfg.n_rel2],
                                    W_R_bcol[:], None, ADD)
            nc.sync.dma_start(rel_newT[:], sb_rn[:])

            # ---- main loop over windows ----
            for w in [w for _ in range(repeat) for w in range(cfg.n_win)]:
                    nt_w = int(plan.t_w[w])
                    # one PSUM bank accumulates every [d, n] contribution:
                    # W_S @ entT_win + rel_partP.T @ C_w + W_O2 @ A_O + W_I2 @ A_I
                    psR = psR_pool.tile([P, P], F32, tag="psR")
                    nc.tensor.matmul(psR[:], lhsT=W_ST[:],
                                     rhs=entT[:, w * P:(w + 1) * P],
                                     start=True, stop=(nt_w == 0))
                    if nt_w:
                        cw = cw_pool.tile([P, 4 * P], F32, tag="cw")
                        nc.sync.dma_start(cw[:], dr["Cw"][w])
                        for blk in range(4):
                            sl = slice(blk * P, (blk + 1) * P)
                            nc.tensor.matmul(psR[:], lhsT=rel_partP[:, sl],
                                             rhs=cw[:, sl], start=False,
                                             stop=False)
                        psA = psA_pool.tile([P, 256], F32, tag="psA")
                        for i in range(nt_w):
                            col = plan.win_t0[w] + i
                            eg = entg_pool.tile([P, D], F32, tag="entg")
                            nc.gpsimd.indirect_dma_start(
                                out=eg[:], out_offset=None,
                                in_=dr["ent_emb"][:],
                                in_offset=bass.IndirectOffsetOnAxis(
                                    ap=ent_idx[:, col:col + 1], axis=0))
                            m2 = m2_pool.tile([P, 256], F32, tag="m2")
                            nc.vector.scalar_tensor_tensor(
                                out=m2[:],
                                in0=iota[:],
                                scalar=dstmod[:, col:col + 1],
                                in1=rd[:, col:col + 1].to_broadcast([P, 256]),
                                op0=ISEQ, op1=MULT)
                            first = (i == 0)
                            last = (i == nt_w - 1)
                            nc.tensor.matmul(psA[:], lhsT=eg[:],
                                             rhs=m2[:], start=first, stop=last)
                        sbA = fin_pool.tile([P, 256], F32, tag="sbA")
                        nc.vector.tensor_copy(sbA[:], psA[:])
                        nc.tensor.matmul(psR[:], lhsT=W_O2T[:],
                                         rhs=sbA[:, :P], start=False,
                                         stop=False)
                        nc.tensor.matmul(psR[:], lhsT=W_I2T[:],
                                         rhs=sbA[:, P:], start=False, stop=True)
                    outw = fin_pool.tile([P, P], F32, tag="outw")
                    nc.vector.tensor_scalar(outw[:], psR[:], b_col[:],
                                            None, ADD)
                    nc.sync.dma_start(ent_newT[:, w * P:(w + 1) * P], outw[:])

    nc.compile()
    return nc


def _run(inputs, cfg: Cfg, trace=False):
    plan, in_maps = _host_prep(inputs, cfg)
    nc = _build_kernel(plan)
    res = run_bass_kernel_spmd(nc, in_maps, core_ids=list(range(cfg.n_cores)),
                               trace=trace)
    ent_rows = []
    for c in range(cfg.n_cores):
        ent_rows.append(res.results[c]["ent_newT"][:, :cfg.npc].T)
    ent_new = np.ascontiguousarray(np.concatenate(ent_rows, axis=0))
    rel_new = np.ascontiguousarray(
        res.results[0]["rel_newT"][:, :cfg.n_rel2].T)
    return (ent_new, rel_new), res


def kernel(**inputs):
    cfg = Cfg()
    (ent_new, rel_new), _ = _run(inputs, cfg)
    return ent_new, rel_new


# revision 23
# speedup vs baseline: 2.2515x; 1.0585x over previous
"""Trainium2 Bass kernel for ExtGNNLayer message passing.

kernel(**inputs) -> (ent_new, rel_new), matching the reference:
    comp_h = concat([rel_emb[etypes], ent_emb[src]])
    msg    = where(etypes < NUM_REL, comp_h @ W_O.T + b_O, comp_h @ W_I.T + b_I)
    h_agg  = segment_mean(msg, dst)
    ent_new = ent_emb @ W_S.T + b_S + h_agg
    rel_new = rel_emb @ W_R.T + b_R

Distribution: edges sharded by destination-node ownership across 8 cores
(6250 nodes/core) -> fully independent cores, no collectives.

Per-core device algorithm (linear algebra reassociated so the per-edge
weight matmuls become per-window):
  - rel_part[t] = rel_emb[t] @ W1_sel(t).T + b_sel(t) precomputed host-side
    (weight folding; W1_sel = first 128 cols of W_O/W_I).
  - Edge rows of ent_emb and rel_part are fetched with batched indirect
    DMA gathers: one SWDGE call per superblock, idx[p, j] -> row at
    dest[p, j*128:(j+1)*128].
  - For each 128-dst-node window, per 128-edge tile (edge e = partition):
      M2[e, n + 128*is_inv_e] = recip_deg[dst_e] * (dst_off_e == n)  (1 DVE op)
      psumA  += entg_tile.T(k=e) @ M2     -> A_O | A_I  [c, 256]
      psumRel+= relg_tile.T(k=e) @ M2     -> R_O | R_I  [d, 256]
  - Window finish:
      psumRel[:, :128] += W_O2 @ A_O + W_I2 @ A_I      (2 matmuls)
      out[d, n] = psumRel_O + psumRel_I + b_S[d] + (W_S @ entT_win)[d, n]
  - Outputs are written transposed [d, n]; the host transposes back.
"""

import dataclasses
import math

import numpy as np

import concourse.bacc as bacc
import concourse.bass as bass
import concourse.mybir as mybir
import concourse.tile as tile
from concourse.bass_utils import run_bass_kernel_spmd

P = 128
F32 = mybir.dt.float32
I32 = mybir.dt.int32
ADD = mybir.AluOpType.add
MULT = mybir.AluOpType.mult
ISEQ = mybir.AluOpType.is_equal


@dataclasses.dataclass
class Cfg:
    n_nodes: int = 50000
    n_rel: int = 200            # etypes < n_rel -> W_O path, else W_I
    dim: int = 128
    n_cores: int = 8
    n_queues: int = 1           # SWDGE queues for the indirect gathers
    sb_w: int = 2               # unused (kept for test compat)

    @property
    def n_rel2(self):
        return 2 * self.n_rel

    @property
    def npc(self):
        return self.n_nodes // self.n_cores

    @property
    def n_win(self):
        return math.ceil(self.npc / P)


@dataclasses.dataclass
class Plan:
    cfg: Cfg
    t_w: np.ndarray           # [n_win] common (max-over-core) tile counts
    win_t0: dict              # w -> first global tile index
    n_tiles: int


def _make_plan(counts, cfg: Cfg) -> Plan:
    """counts: [n_cores, n_win] edge counts."""
    t_w = np.maximum(np.ceil(counts.max(axis=0) / P).astype(np.int64), 0)
    win_t0 = {}
    col = 0
    for w in range(cfg.n_win):
        win_t0[w] = col
        col += int(t_w[w])
    return Plan(cfg=cfg, t_w=t_w, win_t0=win_t0, n_tiles=col)


def _pack_core(plan: Plan, cfg: Cfg, core: int, src, dst, etypes, recip_deg,
               edge_order, block_bounds):
    """Build this core's device arrays ([128, NT] layouts; edge (tile t,
    partition p) at column t, row p)."""
    NT = plan.n_tiles
    ent_idx = np.zeros(NT * P, dtype=np.int32)
    dstmod = np.full(NT * P, -1.0, dtype=np.float32)
    rd = np.zeros(NT * P, dtype=np.float32)

    W = cfg.n_win
    WN = W * P
    TB = 4 * P  # padded relation-type dimension (4 blocks of 128)
    cw_flat = np.zeros(WN * TB, dtype=np.float32)
    for w in range(W):
        nt = int(plan.t_w[w])
        b = core * W + w
        e0, e1 = block_bounds[b], block_bounds[b + 1]
        eids = edge_order[e0:e1]
        cnt = len(eids)
        if cnt == 0:
            continue
        assert nt * P >= cnt
        s = plan.win_t0[w] * P
        ent_idx[s:s + cnt] = src[eids]
        n_off = (dst[eids] - core * cfg.npc - w * P).astype(np.float32)
        is_inv = (etypes[eids] >= cfg.n_rel).astype(np.float32)
        dstmod[s:s + cnt] = n_off + P * is_inv
        rdv = recip_deg[dst[eids]]
        rd[s:s + cnt] = rdv
        # scaled (node, type) counts for this window
        bins = (dst[eids] - core * cfg.npc).astype(np.int64) * TB + etypes[eids]
        np.add.at(cw_flat, bins, rdv)

    # C[w, p=t_local, blk*128+n] = sum_{edges->(w,n)} rd * 1[etype=blk*128+p]
    C = cw_flat.reshape(W, P, 4, P)          # [w, n, blk, t_local]
    C = C.transpose(0, 3, 2, 1).copy()       # [w, t_local, blk, n]

    return dict(
        ent_idx=ent_idx.reshape(NT, P).T.copy(),
        dstmod=dstmod.reshape(NT, P).T.copy(),
        rd=rd.reshape(NT, P).T.copy(),
        Cw=C.reshape(W, P, 4 * P),
    )


def _host_prep(inputs, cfg: Cfg):
    src = np.ascontiguousarray(np.asarray(inputs["src"]).astype(np.int64))
    dst = np.ascontiguousarray(np.asarray(inputs["dst"]).astype(np.int64))
    etypes = np.ascontiguousarray(np.asarray(inputs["etypes"]).astype(np.int64))
    ent_emb = np.asarray(inputs["ent_emb"], dtype=np.float32)
    rel_emb = np.asarray(inputs["rel_emb"], dtype=np.float32)
    W_O_w = np.asarray(inputs["W_O_w"], dtype=np.float32)
    W_O_b = np.asarray(inputs["W_O_b"], dtype=np.float32)
    W_I_w = np.asarray(inputs["W_I_w"], dtype=np.float32)
    W_I_b = np.asarray(inputs["W_I_b"], dtype=np.float32)
    W_S_w = np.asarray(inputs["W_S_w"], dtype=np.float32)
    W_S_b = np.asarray(inputs["W_S_b"], dtype=np.float32)
    W_R_w = np.asarray(inputs["W_R_w"], dtype=np.float32)
    W_R_b = np.asarray(inputs["W_R_b"], dtype=np.float32)

    D = cfg.dim
    deg = np.bincount(dst, minlength=cfg.n_nodes).astype(np.float32)
    recip_deg = (1.0 / np.maximum(deg, 1.0)).astype(np.float32)

    # rel_part[t] = rel_emb[t] @ W1_sel.T + b_sel  (weight folding)
    rel_part_o = rel_emb @ W_O_w[:, :D].T + W_O_b
    rel_part_i = rel_emb @ W_I_w[:, :D].T + W_I_b
    is_inv_t = (np.arange(cfg.n_rel2) >= cfg.n_rel)[:, None]
    rel_part = np.where(is_inv_t, rel_part_i, rel_part_o).astype(np.float32)

    # edge blocks sorted by (core, window), then by src inside each block
    # (ascending gather addresses -> better HBM locality)
    core = dst // cfg.npc
    w = (dst - core * cfg.npc) // P
    key = core * cfg.n_win + w
    edge_order = np.lexsort((src, key))
    counts_flat = np.bincount(key, minlength=cfg.n_cores * cfg.n_win)
    block_bounds = np.concatenate([[0], np.cumsum(counts_flat)])
    counts = counts_flat.reshape(cfg.n_cores, cfg.n_win)

    plan = _make_plan(counts, cfg)

    # rel_partP[p, blk*128+d] = rel_part[blk*128+p, d], zero-padded to 512 rows
    rel_partP = np.zeros((P, 4 * P), dtype=np.float32)
    for blk in range(4):
        rows = rel_part[blk * P:(blk + 1) * P]
        rel_partP[:rows.shape[0], blk * P:blk * P + P] = rows

    WN = cfg.n_win * P
    common = dict(
        ent_emb=ent_emb,
        rel_partP=rel_partP,
        iota=np.broadcast_to(np.arange(256, dtype=np.float32),
                             (P, 256)).copy(),
        W_O2T=W_O_w[:, D:].T.copy(),
        W_I2T=W_I_w[:, D:].T.copy(),
        W_ST=W_S_w.T.copy(),
        b_col=W_S_b[:, None].copy(),
        rel_embT=np.zeros((P, cfg.n_rel2), dtype=np.float32),
        W_RT=W_R_w.T.copy(),
        W_R_bcol=W_R_b[:, None].copy(),
    )
    common["rel_embT"][:rel_emb.shape[1], :] = rel_emb.T

    in_maps = []
    for c in range(cfg.n_cores):
        pc = _pack_core(plan, cfg, c, src, dst, etypes, recip_deg,
                        edge_order, block_bounds)
        entT = np.zeros((P, WN), dtype=np.float32)
        sl = ent_emb[c * cfg.npc:(c + 1) * cfg.npc]
        entT[:, :sl.shape[0]] = sl.T
        m = dict(common)
        m.update(pc)
        m["entT"] = entT
        in_maps.append(m)
    return plan, in_maps


def _build_kernel(plan: Plan, repeat: int = 1, num_devices: int | None = None):
    cfg = plan.cfg
    D = cfg.dim
    NT = plan.n_tiles
    WN = cfg.n_win * P

    nc = bacc.Bacc("TRN2", target_bir_lowering=False, debug=False,
                   num_devices=num_devices or cfg.n_cores,
                   dynamic_dma_scratch_size=65536,
                   num_swdge_queues=cfg.n_queues)

    dr = {}
    def din(name, shape, dt=F32):
        dr[name] = nc.dram_tensor(name, shape, dt, kind="ExternalInput").ap()

    din("ent_emb", [cfg.n_nodes, D])
    din("rel_partP", [P, 4 * P])
    din("Cw", [cfg.n_win, P, 4 * P])
    din("iota", [P, 256])
    din("W_O2T", [D, D])
    din("W_I2T", [D, D])
    din("W_ST", [D, D])
    din("b_col", [D, 1])
    din("rel_embT", [P, cfg.n_rel2])
    din("W_RT", [D, D])
    din("W_R_bcol", [D, 1])
    din("entT", [P, WN])
    din("ent_idx", [P, NT], I32)
    din("dstmod", [P, NT])
    din("rd", [P, NT])

    ent_newT = nc.dram_tensor("ent_newT", [P, WN], F32,
                              kind="ExternalOutput").ap()
    rel_newT = nc.dram_tensor("rel_newT", [P, cfg.n_rel2], F32,
                              kind="ExternalOutput").ap()

    with tile.TileContext(nc) as tc:
        with (
            tc.tile_pool(name="const", bufs=1) as cpool,
            tc.tile_pool(name="entg", bufs=16) as entg_pool,
            tc.tile_pool(name="m2", bufs=12) as m2_pool,
            tc.tile_pool(name="cw", bufs=3) as cw_pool,
            tc.tile_pool(name="fin", bufs=3) as fin_pool,
            tc.tile_pool(name="psA", bufs=3, space="PSUM") as psA_pool,
            tc.tile_pool(name="psR", bufs=3, space="PSUM") as psR_pool,
            tc.tile_pool(name="psrn", bufs=1, space="PSUM") as psrn_pool,
        ):
            def load_const(name, shape, dt=F32):
                t = cpool.tile(shape, dt, tag=name)
                nc.sync.dma_start(t[:], dr[name][:])
                return t

            iota = load_const("iota", [P, 256])
            W_O2T = load_const("W_O2T", [D, D])
            W_I2T = load_const("W_I2T", [D, D])
            W_ST = load_const("W_ST", [D, D])
            b_col = load_const("b_col", [D, 1])
            rel_embT = load_const("rel_embT", [P, cfg.n_rel2])
            W_RT = load_const("W_RT", [D, D])
            W_R_bcol = load_const("W_R_bcol", [D, 1])
            rel_partP = load_const("rel_partP", [P, 4 * P])
            entT = load_const("entT", [P, WN])
            ent_idx = load_const("ent_idx", [P, NT], I32)
            dstmod = load_const("dstmod", [P, NT])
            rd = load_const("rd", [P, NT])

            # ---- rel_new = (W_R @ rel_embT + b) transposed ----
            ps_rn = psrn_pool.tile([P, 512], F32, tag="psrn")
            n_rt = math.ceil(cfg.n_rel2 / P)
            for j in range(n_rt):
                c0 = j * P
                c1 = min(c0 + P, cfg.n_rel2)
                nc.tensor.matmul(ps_rn[:, c0:c1], lhsT=W_RT[:],
                                 rhs=rel_embT[:, c0:c1], start=True, stop=True)
            sb_rn = fin_pool.tile([P, cfg.n_rel2], F32, tag="sbrn")
            nc.vector.tensor_scalar(sb_rn[:], ps_rn[:, :cfg.n_rel2],
                                    W_R_bcol[:], None, ADD)
            nc.sync.dma_start(rel_newT[:], sb_rn[:])

            # ---- main loop over windows ----
            for w in [w for _ in range(repeat) for w in range(cfg.n_win)]:
                    nt_w = int(plan.t_w[w])
                    # one PSUM bank accumulates every [d, n] contribution:
                    # W_S @ entT_win + rel_partP.T @ C_w + W_O2 @ A_O + W_I2 @ A_I
                    psR = psR_pool.tile([P, P], F32, tag="psR")
                    nc.tensor.matmul(psR[:], lhsT=W_ST[:],
                                     rhs=entT[:, w * P:(w + 1) * P],
                                     start=True, stop=(nt_w == 0))
                    if nt_w:
                        cw = cw_pool.tile([P, 4 * P], F32, tag="cw")
                        nc.sync.dma_start(cw[:], dr["Cw"][w])
                        for blk in range(4):
                            sl = slice(blk * P, (blk + 1) * P)
                            nc.tensor.matmul(psR[:], lhsT=rel_partP[:, sl],
                                             rhs=cw[:, sl], start=False,
                                             stop=False)
                        psA = psA_pool.tile([P, 256], F32, tag="psA")
                        for i in range(nt_w):
                            col = plan.win_t0[w] + i
                            eg = entg_pool.tile([P, D], F32, tag="entg")
                            g = nc.gpsimd.indirect_dma_start(
                                out=eg[:], out_offset=None,
                                in_=dr["ent_emb"][:],
                                in_offset=bass.IndirectOffsetOnAxis(
                                    ap=ent_idx[:, col:col + 1], axis=0))
                            if cfg.n_queues > 1:
                                q = col % cfg.n_queues
                                if q:
                                    g.ins.queue = f"qPoolDynamic{q}"
                            m2 = m2_pool.tile([P, 256], F32, tag="m2")
                            nc.vector.scalar_tensor_tensor(
                                out=m2[:],
                                in0=iota[:],
                                scalar=dstmod[:, col:col + 1],
                                in1=rd[:, col:col + 1].to_broadcast([P, 256]),
                                op0=ISEQ, op1=MULT)
                            first = (i == 0)
                            last = (i == nt_w - 1)
                            nc.tensor.matmul(psA[:], lhsT=eg[:],
                                             rhs=m2[:], start=first, stop=last)
                        sbA = fin_pool.tile([P, 256], F32, tag="sbA")
                        nc.vector.tensor_copy(sbA[:], psA[:])
                        nc.tensor.matmul(psR[:], lhsT=W_O2T[:],
                                         rhs=sbA[:, :P], start=False,
                                         stop=False)
                        nc.tensor.matmul(psR[:], lhsT=W_I2T[:],
                                         rhs=sbA[:, P:], start=False, stop=True)
                    outw = fin_pool.tile([P, P], F32, tag="outw")
                    nc.vector.tensor_scalar(outw[:], psR[:], b_col[:],
                                            None, ADD)
                    nc.sync.dma_start(ent_newT[:, w * P:(w + 1) * P], outw[:])

    nc.compile()
    return nc


def _run(inputs, cfg: Cfg, trace=False):
    plan, in_maps = _host_prep(inputs, cfg)
    nc = _build_kernel(plan)
    res = run_bass_kernel_spmd(nc, in_maps, core_ids=list(range(cfg.n_cores)),
                               trace=trace)
    ent_rows = []
    for c in range(cfg.n_cores):
        ent_rows.append(res.results[c]["ent_newT"][:, :cfg.npc].T)
    ent_new = np.ascontiguousarray(np.concatenate(ent_rows, axis=0))
    rel_new = np.ascontiguousarray(
        res.results[0]["rel_newT"][:, :cfg.n_rel2].T)
    return (ent_new, rel_new), res


def kernel(**inputs):
    cfg = Cfg()
    (ent_new, rel_new), _ = _run(inputs, cfg)
    return ent_new, rel_new
